# revision 2
# baseline (speedup 1.0000x reference)
"""Trainium2 Bass kernel v2 for ContrastiveGNN (3x GCNConv + 2-layer projector).

Sharding: nodes partitioned across 8 NeuronCores (degree-sorted bands of
1024 = 8 cores x 128 partitions). Bands are grouped (<=8 bands, <=Q slots)
with a uniform per-group max in-degree Kbar so each group's in-edge sources
gather in ONE batched indirect DMA (dest AP rearranged slot-major so the
descriptor order matches the DGE's partition-fast offset consumption).
Per layer: grouped matmul (h = act @ W) -> p' = h*r -> AllGather p' into a
full table -> per group: batched gather + single strided reduce + fused
epilogue: act = relu((sum + p')*r + b). Projector MLP fused per group.
"""
import numpy as np

import concourse.bacc as bacc
import concourse.bass as bass
import concourse.mybir as mybir
import concourse.tile as tile
from concourse.masks import make_identity

N = 100000
E = 1600000
IN_D, HID, OUT_D = 256, 64, 64
N_CORES = 8
P = 128
Q = 160  # max slots per group gather
F32 = mybir.dt.float32
BF16 = mybir.dt.bfloat16


def _plan(edge_index: np.ndarray, n: int):
    band = N_CORES * P
    n_bands = (n + band - 1) // band
    rpc = n_bands * P
    ntot = N_CORES * rpc

    src = np.asarray(edge_index[0], dtype=np.int64)
    dst = np.asarray(edge_index[1], dtype=np.int64)
    ne = len(src)
    deg = np.bincount(dst, minlength=n).astype(np.float64) + 1.0

    order = np.argsort(-deg, kind="stable")
    i = np.arange(n)
    g, s = i // band, i % band
    c, p = s // P, s % P
    new_id = np.empty(n, dtype=np.int64)
    new_id[order] = c * rpc + g * P + p

    nsrc = new_id[src]
    ndst = new_id[dst]

    cnt = np.bincount(ndst, minlength=ntot)
    Kb = cnt.reshape(N_CORES, n_bands, P).max(axis=(0, 2)).astype(np.int64)
    Kb = np.maximum(Kb, 1)

    # greedy grouping: <=8 bands per group, <=Q slots (G*Kbar)
    groups = []
    b = 0
    while b < n_bands:
        kbar, gc = int(Kb[b]), 1
        while (gc < 8 and b + gc < n_bands
               and (gc + 1) * max(kbar, int(Kb[b + gc])) <= Q):
            kbar = max(kbar, int(Kb[b + gc]))
            gc += 1
        groups.append((b, gc, kbar))
        b += gc
    COL0 = np.zeros(len(groups) + 1, np.int64)
    for gi, (_, gc, kbar) in enumerate(groups):
        COL0[gi + 1] = COL0[gi] + gc * kbar
    SLOTS = int(COL0[-1])

    # per-band -> (group id, base column)
    band_col0 = np.zeros(n_bands, np.int64)
    band_kbar = np.zeros(n_bands, np.int64)
    for gi, (b0, gc, kbar) in enumerate(groups):
        for j in range(gc):
            band_col0[b0 + j] = COL0[gi] + j * kbar
            band_kbar[b0 + j] = kbar

    pad_row = ntot - 1
    assert n < ntot, "need at least one dummy node for the zero row"

    eo = np.argsort(ndst, kind="stable")
    sdst, ssrc = ndst[eo], nsrc[eo]
    first = np.searchsorted(sdst, sdst, side="left")
    kidx = np.arange(ne) - first
    ec = sdst // rpc
    loc = sdst % rpc
    eg = loc // P
    ep = loc % P
    col = band_col0[eg] + kidx
    sidx = np.full((N_CORES, P, SLOTS), pad_row, dtype=np.int32)
    sidx[ec, ep, col] = ssrc.astype(np.int32)

    # r (deg_rsqrt) replicated across the 64 feature cols, band-major
    dr = (1.0 / np.sqrt(deg)).astype(np.float32)
    drn = np.zeros(ntot, np.float32)
    drn[new_id] = dr
    # drn layout: core-major [c, band, p]; per core want [P, n_bands*64]
    rw = np.zeros((N_CORES, P, n_bands * 64), np.float32)
    per_core = drn.reshape(N_CORES, n_bands, P)
    for cc in range(N_CORES):
        rw[cc] = np.repeat(per_core[cc].T[:, :, None], 64,
                           axis=2).reshape(P, n_bands * 64)

    return dict(new_id=new_id, groups=groups, COL0=[int(x) for x in COL0],
                SLOTS=SLOTS, sidx=sidx, rw=rw, pad_row=pad_row,
                n_bands=n_bands, rpc=rpc, ntot=ntot, n=n)


def _build(pl, in_d=IN_D, table_bf16=True, no_gather=False, no_coll=False,
           no_reduce=False):
    groups, COL0, SLOTS = pl["groups"], pl["COL0"], pl["SLOTS"]
    n_bands, rpc, ntot = pl["n_bands"], pl["rpc"], pl["ntot"]
    TDT = BF16 if table_bf16 else F32

    nc = bacc.Bacc("TRN2", target_bir_lowering=False, debug=False,
                   num_devices=N_CORES)
    xT = nc.dram_tensor("xT", [in_d, rpc], F32, kind="ExternalInput")
    W1 = nc.dram_tensor("W1", [in_d, HID], F32, kind="ExternalInput")
    W2 = nc.dram_tensor("W2", [HID, HID], F32, kind="ExternalInput")
    W3 = nc.dram_tensor("W3", [HID, OUT_D], F32, kind="ExternalInput")
    Pw1 = nc.dram_tensor("Pw1", [OUT_D, HID], F32, kind="ExternalInput")
    Pw2 = nc.dram_tensor("Pw2", [HID, OUT_D], F32, kind="ExternalInput")
    brep = nc.dram_tensor("brep", [P, 5 * 512], F32, kind="ExternalInput")
    rwt = nc.dram_tensor("rwt", [P, n_bands * 64], F32, kind="ExternalInput")
    sidx = nc.dram_tensor("sidx", [P, SLOTS], mybir.dt.int32,
                          kind="ExternalInput")
    z = nc.dram_tensor("z", [rpc, 64], F32, kind="ExternalOutput")

    kchunks = (in_d + P - 1) // P
    mult = mybir.AluOpType.mult

    with tile.TileContext(nc) as tc:
        with (
            tc.tile_pool(name="const", bufs=1) as cpool,
            tc.tile_pool(name="psb", bufs=2) as ppool,
            tc.tile_pool(name="work", bufs=3) as wpool,
            tc.tile_pool(name="xs", bufs=2) as xpool,
            tc.tile_pool(name="gbuf", bufs=2) as gpool,
            tc.tile_pool(name="psmm", bufs=4, space="PSUM") as psmm,
            tc.tile_pool(name="pstr", bufs=4, space="PSUM") as pstr,
            tc.tile_pool(name="dram", bufs=2, space="DRAM") as dpool,
        ):
            w1s = []
            for kc in range(kchunks):
                kp = min(P, in_d - kc * P)
                wc = cpool.tile([kp, HID], F32, tag=f"w1_{kc}")
                nc.sync.dma_start(wc[:], W1[kc * P:kc * P + kp, :])
                w1s.append(wc)
            # duplicate 64-row weights into both partition halves so paired
            # transposes can matmul from base partition 0 or 64
            w2 = cpool.tile([2 * HID, HID], F32, tag="w2")
            w3 = cpool.tile([2 * HID, OUT_D], F32, tag="w3")
            pw1 = cpool.tile([2 * OUT_D, HID], F32, tag="pw1")
            pw2 = cpool.tile([2 * HID, OUT_D], F32, tag="pw2")
            for wt, wsrc in ((w2, W2), (w3, W3), (pw1, Pw1), (pw2, Pw2)):
                nc.sync.dma_start(wt[:HID, :], wsrc[:])
                nc.sync.dma_start(wt[HID:, :], wsrc[:])
            bsb = cpool.tile([P, 5 * 512], F32, tag="bsb")
            rsb = cpool.tile([P, n_bands * 64], F32, tag="rsb")
            isb = cpool.tile([P, SLOTS], mybir.dt.int32, tag="isb")
            nc.sync.dma_start(bsb[:], brep[:])
            nc.sync.dma_start(rsb[:], rwt[:])
            nc.sync.dma_start(isb[:], sidx[:])
            ident = cpool.tile([P, P], F32, tag="ident")
            make_identity(nc, ident[:])

            def bias(L, width):
                return bsb[:, L * 512:L * 512 + width]

            n_grp = len(groups)
            SPLIT = n_grp  # chunked AG disabled (Shared single-writer rule)

            def all_gather(ag_t, table_t, r0, r1):
                if no_coll or r0 >= r1:
                    return
                nc.gpsimd.collective_compute(
                    "AllGather", mybir.AluOpType.bypass,
                    replica_groups=[list(range(N_CORES))],
                    ins=[ag_t.opt()], outs=[table_t.opt()],
                )

            def store_rows(dst_t, b0, G, pb):
                nc.sync.dma_start(
                    dst_t[b0 * P:(b0 + G) * P, :].rearrange(
                        "(tt p) f -> p tt f", p=P),
                    pb[:].rearrange("p (tt f) -> p tt f", f=64))

            def mm_pairs(ps, a, G, w):
                # per-band transpose + matmul, operands at base partition 0
                for j in range(G):
                    tr = pstr.tile([64, P], F32, tag="tr")
                    nc.tensor.transpose(tr[:], a[:, j * 64:(j + 1) * 64],
                                        ident[:])
                    lh = wpool.tile([64, P], F32, tag="lh")
                    nc.vector.tensor_copy(lh[:], tr[:])
                    nc.tensor.matmul(ps[:, j * 64:(j + 1) * 64],
                                     lh[:], w[:HID, :],
                                     start=True, stop=True)

            # ---- layer-1 matmul phase (streamed xT) ----
            p_cur = ppool.tile([P, n_bands * 64], F32, tag="psb")
            ag = dpool.tile([rpc, 64], TDT, tag="ag")
            table = dpool.tile([ntot, 64], TDT, addr_space="Shared",
                               tag="table")
            for g0i, (b0, G, kbar) in enumerate(groups):
                cols = slice(b0 * 64, (b0 + G) * 64)
                xcs = []
                for kc in range(kchunks):
                    xc = xpool.tile([P, G * P], F32, tag=f"xs{kc}")
                    nc.sync.dma_start(
                        xc[:], xT[kc * P:(kc + 1) * P, b0 * P:(b0 + G) * P])
                    xcs.append(xc)
                ps = psmm.tile([P, G * 64], F32, tag="ps")
                for j in range(G):
                    for kc in range(kchunks):
                        nc.tensor.matmul(ps[:, j * 64:(j + 1) * 64],
                                         xcs[kc][:, j * P:(j + 1) * P],
                                         w1s[kc][:],
                                         start=(kc == 0),
                                         stop=(kc == kchunks - 1))
                nc.vector.tensor_tensor(p_cur[:, cols], ps[:], rsb[:, cols],
                                        op=mult)
                pb = wpool.tile([P, G * 64], TDT, tag="pbf")
                nc.vector.tensor_copy(pb[:], p_cur[:, cols])
                store_rows(ag, b0, G, pb)
            all_gather(ag, table, 0, rpc)

            # ---- 3 aggregation phases ----
            for L in range(1, 4):
                w_next = {1: w2, 2: w3}.get(L)
                if w_next is not None:
                    p_next = ppool.tile([P, n_bands * 64], F32, tag="psb")
                    ag_next = dpool.tile([rpc, 64], TDT, tag="ag")
                    table_next = dpool.tile([ntot, 64], TDT,
                                            addr_space="Shared", tag="table")
                for gi, (b0, G, kbar) in enumerate(groups):
                    S = G * kbar
                    c0 = COL0[gi]
                    cols = slice(b0 * 64, (b0 + G) * 64)
                    acc = wpool.tile([P, G * 64], F32, tag="acc")
                    if no_gather:
                        nc.vector.memset(acc[:], 0.0)
                    else:
                        gt = gpool.tile([P, S, 64], TDT, tag="gt")
                        for k in range(S):
                            nc.gpsimd.indirect_dma_start(
                                out=gt[:, k], out_offset=None, in_=table[:],
                                in_offset=bass.IndirectOffsetOnAxis(
                                    ap=isb[:, c0 + k:c0 + k + 1], axis=0),
                            )
                        if no_reduce:
                            nc.vector.memset(acc[:], 0.0)
                        else:
                            for b in range(G):
                                nc.vector.reduce_sum(
                                    out=acc[:, b * 64:(b + 1) * 64],
                                    in_=gt[:, b * kbar:(b + 1) * kbar, :]
                                    .rearrange("p k f -> p f k"),
                                    axis=mybir.AxisListType.X)
                    t1 = wpool.tile([P, G * 64], F32, tag="t1")
                    nc.vector.tensor_add(t1[:], acc[:], p_cur[:, cols])
                    nc.vector.tensor_tensor(t1[:], t1[:], rsb[:, cols],
                                            op=mult)
                    nc.vector.tensor_add(t1[:], t1[:], bias(L - 1, G * 64))
                    a = wpool.tile([P, G * 64], F32, tag="a")
                    nc.scalar.activation(a[:], t1[:],
                                         mybir.ActivationFunctionType.Relu)
                    if w_next is not None:
                        ps = psmm.tile([P, G * 64], F32, tag="ps")
                        mm_pairs(ps, a, G, w_next)
                        nc.vector.tensor_tensor(p_next[:, cols], ps[:],
                                                rsb[:, cols], op=mult)
                        pb = wpool.tile([P, G * 64], TDT, tag="pbf")
                        nc.vector.tensor_copy(pb[:], p_next[:, cols])
                        store_rows(ag_next, b0, G, pb)
                    else:
                        # projector: q = relu(a @ Pw1 + Pb1); z = q @ Pw2 + Pb2
                        psq = psmm.tile([P, G * 64], F32, tag="ps")
                        mm_pairs(psq, a, G, pw1)
                        q0 = wpool.tile([P, G * 64], F32, tag="t1")
                        nc.vector.tensor_add(q0[:], psq[:], bias(3, G * 64))
                        q = wpool.tile([P, G * 64], F32, tag="q")
                        nc.scalar.activation(q[:], q0[:],
                                             mybir.ActivationFunctionType.Relu)
                        psz = psmm.tile([P, G * 64], F32, tag="ps")
                        mm_pairs(psz, q, G, pw2)
                        zt = wpool.tile([P, G * 64], F32, tag="zt")
                        nc.vector.tensor_add(zt[:], psz[:], bias(4, G * 64))
                        store_rows(z, b0, G, zt)
                if w_next is not None:
                    all_gather(ag_next, table_next, 0, rpc)
                    p_cur = p_next
                    ag = ag_next
                    table = table_next

    nc.compile()
    return nc


def _in_maps(inputs, pl, in_d=IN_D):
    x = np.asarray(inputs["x"], np.float32)
    new_id = pl["new_id"]
    rpc, ntot = pl["rpc"], pl["ntot"]
    xn = np.zeros((ntot, in_d), np.float32)
    xn[new_id] = x
    brep = np.tile(
        np.concatenate([
            np.tile(np.asarray(inputs[k], np.float32), 8)
            for k in ("b1", "b2", "b3", "Pb1", "Pb2")
        ])[None, :], (P, 1))
    common = dict(
        W1=np.asarray(inputs["W1"], np.float32),
        W2=np.asarray(inputs["W2"], np.float32),
        W3=np.asarray(inputs["W3"], np.float32),
        Pw1=np.asarray(inputs["Pw1"], np.float32),
        Pw2=np.asarray(inputs["Pw2"], np.float32),
        brep=brep,
    )
    maps = []
    for c in range(N_CORES):
        xc = xn[c * rpc:(c + 1) * rpc]
        maps.append(dict(
            xT=np.ascontiguousarray(xc.T),
            rwt=pl["rw"][c],
            sidx=pl["sidx"][c],
            **common,
        ))
    return maps


def build_all(inputs, n=None, in_d=IN_D, **bkw):
    x = np.asarray(inputs["x"])
    n = x.shape[0] if n is None else n
    pl = _plan(np.asarray(inputs["edge_index"]), n)
    nc = _build(pl, in_d=in_d, **bkw)
    maps = _in_maps(inputs, pl, in_d=in_d)
    return nc, maps, pl


def postprocess(results, pl):
    z_new = np.concatenate([results[c]["z"] for c in range(N_CORES)], axis=0)
    return np.ascontiguousarray(z_new[pl["new_id"]]).astype(np.float32)


def kernel(**inputs) -> np.ndarray:
    from concourse.bass_utils import run_bass_kernel_spmd
    nc, maps, pl = build_all(inputs)
    res = run_bass_kernel_spmd(nc, maps, core_ids=list(range(N_CORES)))
    return postprocess(res.results, pl)


# revision 3
# speedup vs baseline: 2.5684x; 2.5684x over previous
"""Trainium2 Bass kernel v2 for ContrastiveGNN (3x GCNConv + 2-layer projector).

Sharding: nodes partitioned across 8 NeuronCores (degree-sorted bands of
1024 = 8 cores x 128 partitions). Bands are grouped (<=8 bands, <=Q slots)
with a uniform per-group max in-degree Kbar so each group's in-edge sources
gather in ONE batched indirect DMA (dest AP rearranged slot-major so the
descriptor order matches the DGE's partition-fast offset consumption).
Per layer: grouped matmul (h = act @ W) -> p' = h*r -> AllGather p' into a
full table -> per group: batched gather + single strided reduce + fused
epilogue: act = relu((sum + p')*r + b). Projector MLP fused per group.
"""
import numpy as np

import concourse.bacc as bacc
import concourse.bass as bass
import concourse.mybir as mybir
import concourse.tile as tile
from concourse.masks import make_identity

N = 100000
E = 1600000
IN_D, HID, OUT_D = 256, 64, 64
N_CORES = 8
P = 128
Q = 160  # max slots per group gather
F32 = mybir.dt.float32
BF16 = mybir.dt.bfloat16


def _plan(edge_index: np.ndarray, n: int):
    band = N_CORES * P
    n_bands = (n + band - 1) // band
    rpc = n_bands * P
    ntot = N_CORES * rpc

    src = np.asarray(edge_index[0], dtype=np.int64)
    dst = np.asarray(edge_index[1], dtype=np.int64)
    ne = len(src)
    deg = np.bincount(dst, minlength=n).astype(np.float64) + 1.0

    order = np.argsort(-deg, kind="stable")
    i = np.arange(n)
    g, s = i // band, i % band
    c, p = s // P, s % P
    new_id = np.empty(n, dtype=np.int64)
    new_id[order] = c * rpc + g * P + p

    nsrc = new_id[src]
    ndst = new_id[dst]

    cnt = np.bincount(ndst, minlength=ntot)
    Kb = cnt.reshape(N_CORES, n_bands, P).max(axis=(0, 2)).astype(np.int64)
    Kb = np.maximum(Kb, 1)

    # greedy grouping: <=8 bands per group, per-band K, sum K <= Q slots
    groups = []  # (b0, gc, [K per band])
    b = 0
    while b < n_bands:
        ks = [int(Kb[b])]
        gc = 1
        while (gc < 8 and b + gc < n_bands
               and sum(ks) + int(Kb[b + gc]) <= Q):
            ks.append(int(Kb[b + gc]))
            gc += 1
        groups.append((b, gc, ks))
        b += gc
    COL0 = np.zeros(len(groups) + 1, np.int64)
    for gi, (_, gc, ks) in enumerate(groups):
        COL0[gi + 1] = COL0[gi] + sum(ks)
    SLOTS = int(COL0[-1])

    # per-band -> base column
    band_col0 = np.zeros(n_bands, np.int64)
    for gi, (b0, gc, ks) in enumerate(groups):
        off = 0
        for j in range(gc):
            band_col0[b0 + j] = COL0[gi] + off
            off += ks[j]

    pad_row = ntot - 1
    assert n < ntot, "need at least one dummy node for the zero row"

    eo = np.argsort(ndst, kind="stable")
    sdst, ssrc = ndst[eo], nsrc[eo]
    first = np.searchsorted(sdst, sdst, side="left")
    kidx = np.arange(ne) - first
    ec = sdst // rpc
    loc = sdst % rpc
    eg = loc // P
    ep = loc % P
    col = band_col0[eg] + kidx
    sidx = np.full((N_CORES, P, SLOTS), pad_row, dtype=np.int32)
    sidx[ec, ep, col] = ssrc.astype(np.int32)

    # r (deg_rsqrt) replicated across the 64 feature cols, band-major
    dr = (1.0 / np.sqrt(deg)).astype(np.float32)
    drn = np.zeros(ntot, np.float32)
    drn[new_id] = dr
    # drn layout: core-major [c, band, p]; per core want [P, n_bands*64]
    rw = np.zeros((N_CORES, P, n_bands * 64), np.float32)
    per_core = drn.reshape(N_CORES, n_bands, P)
    for cc in range(N_CORES):
        rw[cc] = np.repeat(per_core[cc].T[:, :, None], 64,
                           axis=2).reshape(P, n_bands * 64)

    return dict(new_id=new_id, groups=groups, COL0=[int(x) for x in COL0],
                SLOTS=SLOTS, sidx=sidx, rw=rw, pad_row=pad_row,
                n_bands=n_bands, rpc=rpc, ntot=ntot, n=n)


def _build(pl, in_d=IN_D, table_bf16=True, no_gather=False, no_coll=False,
           no_reduce=False):
    groups, COL0, SLOTS = pl["groups"], pl["COL0"], pl["SLOTS"]
    n_bands, rpc, ntot = pl["n_bands"], pl["rpc"], pl["ntot"]
    TDT = BF16 if table_bf16 else F32

    nc = bacc.Bacc("TRN2", target_bir_lowering=False, debug=False,
                   num_devices=N_CORES)
    xT = nc.dram_tensor("xT", [in_d, rpc], F32, kind="ExternalInput")
    W1 = nc.dram_tensor("W1", [in_d, HID], F32, kind="ExternalInput")
    W2 = nc.dram_tensor("W2", [HID, HID], F32, kind="ExternalInput")
    W3 = nc.dram_tensor("W3", [HID, OUT_D], F32, kind="ExternalInput")
    Pw1 = nc.dram_tensor("Pw1", [OUT_D, HID], F32, kind="ExternalInput")
    Pw2 = nc.dram_tensor("Pw2", [HID, OUT_D], F32, kind="ExternalInput")
    brep = nc.dram_tensor("brep", [P, 5 * 512], F32, kind="ExternalInput")
    rwt = nc.dram_tensor("rwt", [P, n_bands * 64], F32, kind="ExternalInput")
    sidx = nc.dram_tensor("sidx", [P, SLOTS], mybir.dt.int32,
                          kind="ExternalInput")
    z = nc.dram_tensor("z", [rpc, 64], F32, kind="ExternalOutput")

    kchunks = (in_d + P - 1) // P
    mult = mybir.AluOpType.mult

    with tile.TileContext(nc) as tc:
        with (
            tc.tile_pool(name="const", bufs=1) as cpool,
            tc.tile_pool(name="psb", bufs=2) as ppool,
            tc.tile_pool(name="work", bufs=3) as wpool,
            tc.tile_pool(name="xs", bufs=2) as xpool,
            tc.tile_pool(name="gbuf", bufs=2) as gpool,
            tc.tile_pool(name="psmm", bufs=4, space="PSUM") as psmm,
            tc.tile_pool(name="pstr", bufs=4, space="PSUM") as pstr,
            tc.tile_pool(name="dram", bufs=2, space="DRAM") as dpool,
        ):
            w1s = []
            for kc in range(kchunks):
                kp = min(P, in_d - kc * P)
                wc = cpool.tile([kp, HID], F32, tag=f"w1_{kc}")
                nc.sync.dma_start(wc[:], W1[kc * P:kc * P + kp, :])
                w1s.append(wc)
            # duplicate 64-row weights into both partition halves so paired
            # transposes can matmul from base partition 0 or 64
            w2 = cpool.tile([2 * HID, HID], F32, tag="w2")
            w3 = cpool.tile([2 * HID, OUT_D], F32, tag="w3")
            pw1 = cpool.tile([2 * OUT_D, HID], F32, tag="pw1")
            pw2 = cpool.tile([2 * HID, OUT_D], F32, tag="pw2")
            for wt, wsrc in ((w2, W2), (w3, W3), (pw1, Pw1), (pw2, Pw2)):
                nc.sync.dma_start(wt[:HID, :], wsrc[:])
                nc.sync.dma_start(wt[HID:, :], wsrc[:])
            bsb = cpool.tile([P, 5 * 512], F32, tag="bsb")
            rsb = cpool.tile([P, n_bands * 64], F32, tag="rsb")
            isb = cpool.tile([P, SLOTS], mybir.dt.int32, tag="isb")
            nc.sync.dma_start(bsb[:], brep[:])
            nc.sync.dma_start(rsb[:], rwt[:])
            nc.sync.dma_start(isb[:], sidx[:])
            ident = cpool.tile([P, P], F32, tag="ident")
            make_identity(nc, ident[:])

            def bias(L, width):
                return bsb[:, L * 512:L * 512 + width]

            n_grp = len(groups)
            SPLIT = n_grp  # chunked AG disabled (Shared single-writer rule)

            def all_gather(ag_t, table_t, r0, r1):
                if no_coll or r0 >= r1:
                    return
                nc.gpsimd.collective_compute(
                    "AllGather", mybir.AluOpType.bypass,
                    replica_groups=[list(range(N_CORES))],
                    ins=[ag_t.opt()], outs=[table_t.opt()],
                )

            def store_rows(dst_t, b0, G, pb):
                nc.sync.dma_start(
                    dst_t[b0 * P:(b0 + G) * P, :].rearrange(
                        "(tt p) f -> p tt f", p=P),
                    pb[:].rearrange("p (tt f) -> p tt f", f=64))

            def mm_pairs(ps, a, G, w):
                # per-band transpose + matmul, operands at base partition 0
                for j in range(G):
                    tr = pstr.tile([64, P], F32, tag="tr")
                    nc.tensor.transpose(tr[:], a[:, j * 64:(j + 1) * 64],
                                        ident[:])
                    lh = wpool.tile([64, P], F32, tag="lh")
                    nc.vector.tensor_copy(lh[:], tr[:])
                    nc.tensor.matmul(ps[:, j * 64:(j + 1) * 64],
                                     lh[:], w[:HID, :],
                                     start=True, stop=True)

            # ---- layer-1 matmul phase (streamed xT) ----
            p_cur = ppool.tile([P, n_bands * 64], F32, tag="psb")
            ag = dpool.tile([rpc, 64], TDT, tag="ag")
            table = dpool.tile([ntot, 64], TDT, addr_space="Shared",
                               tag="table")
            for g0i, (b0, G, ks) in enumerate(groups):
                cols = slice(b0 * 64, (b0 + G) * 64)
                xcs = []
                for kc in range(kchunks):
                    xc = xpool.tile([P, G * P], F32, tag=f"xs{kc}")
                    nc.sync.dma_start(
                        xc[:], xT[kc * P:(kc + 1) * P, b0 * P:(b0 + G) * P])
                    xcs.append(xc)
                ps = psmm.tile([P, G * 64], F32, tag="ps")
                for j in range(G):
                    for kc in range(kchunks):
                        nc.tensor.matmul(ps[:, j * 64:(j + 1) * 64],
                                         xcs[kc][:, j * P:(j + 1) * P],
                                         w1s[kc][:],
                                         start=(kc == 0),
                                         stop=(kc == kchunks - 1))
                nc.vector.tensor_tensor(p_cur[:, cols], ps[:], rsb[:, cols],
                                        op=mult)
                pb = wpool.tile([P, G * 64], TDT, tag="pbf")
                nc.vector.tensor_copy(pb[:], p_cur[:, cols])
                store_rows(ag, b0, G, pb)
            all_gather(ag, table, 0, rpc)

            # ---- 3 aggregation phases ----
            for L in range(1, 4):
                w_next = {1: w2, 2: w3}.get(L)
                if w_next is not None:
                    p_next = ppool.tile([P, n_bands * 64], F32, tag="psb")
                    ag_next = dpool.tile([rpc, 64], TDT, tag="ag")
                    table_next = dpool.tile([ntot, 64], TDT,
                                            addr_space="Shared", tag="table")
                for gi, (b0, G, ks) in enumerate(groups):
                    S = sum(ks)
                    c0 = COL0[gi]
                    cols = slice(b0 * 64, (b0 + G) * 64)
                    acc = wpool.tile([P, G * 64], F32, tag="acc")
                    if no_gather:
                        nc.vector.memset(acc[:], 0.0)
                    else:
                        gt = gpool.tile([P, S, 64], TDT, tag="gt")
                        for k in range(S):
                            nc.gpsimd.indirect_dma_start(
                                out=gt[:, k], out_offset=None, in_=table[:],
                                in_offset=bass.IndirectOffsetOnAxis(
                                    ap=isb[:, c0 + k:c0 + k + 1], axis=0),
                            )
                        if no_reduce:
                            nc.vector.memset(acc[:], 0.0)
                        else:
                            off = 0
                            for b in range(G):
                                kk = ks[b]
                                nc.vector.reduce_sum(
                                    out=acc[:, b * 64:(b + 1) * 64],
                                    in_=gt[:, off:off + kk, :]
                                    .rearrange("p k f -> p f k"),
                                    axis=mybir.AxisListType.X)
                                off += kk
                    t1 = wpool.tile([P, G * 64], F32, tag="t1")
                    nc.vector.tensor_add(t1[:], acc[:], p_cur[:, cols])
                    nc.vector.tensor_tensor(t1[:], t1[:], rsb[:, cols],
                                            op=mult)
                    nc.vector.tensor_add(t1[:], t1[:], bias(L - 1, G * 64))
                    a = wpool.tile([P, G * 64], F32, tag="a")
                    nc.scalar.activation(a[:], t1[:],
                                         mybir.ActivationFunctionType.Relu)
                    if w_next is not None:
                        ps = psmm.tile([P, G * 64], F32, tag="ps")
                        mm_pairs(ps, a, G, w_next)
                        nc.vector.tensor_tensor(p_next[:, cols], ps[:],
                                                rsb[:, cols], op=mult)
                        pb = wpool.tile([P, G * 64], TDT, tag="pbf")
                        nc.vector.tensor_copy(pb[:], p_next[:, cols])
                        store_rows(ag_next, b0, G, pb)
                    else:
                        # projector: q = relu(a @ Pw1 + Pb1); z = q @ Pw2 + Pb2
                        psq = psmm.tile([P, G * 64], F32, tag="ps")
                        mm_pairs(psq, a, G, pw1)
                        q0 = wpool.tile([P, G * 64], F32, tag="t1")
                        nc.vector.tensor_add(q0[:], psq[:], bias(3, G * 64))
                        q = wpool.tile([P, G * 64], F32, tag="q")
                        nc.scalar.activation(q[:], q0[:],
                                             mybir.ActivationFunctionType.Relu)
                        psz = psmm.tile([P, G * 64], F32, tag="ps")
                        mm_pairs(psz, q, G, pw2)
                        zt = wpool.tile([P, G * 64], F32, tag="zt")
                        nc.vector.tensor_add(zt[:], psz[:], bias(4, G * 64))
                        store_rows(z, b0, G, zt)
                if w_next is not None:
                    all_gather(ag_next, table_next, 0, rpc)
                    p_cur = p_next
                    ag = ag_next
                    table = table_next

    nc.compile()
    return nc


def _in_maps(inputs, pl, in_d=IN_D):
    x = np.asarray(inputs["x"], np.float32)
    new_id = pl["new_id"]
    rpc, ntot = pl["rpc"], pl["ntot"]
    xn = np.zeros((ntot, in_d), np.float32)
    xn[new_id] = x
    brep = np.tile(
        np.concatenate([
            np.tile(np.asarray(inputs[k], np.float32), 8)
            for k in ("b1", "b2", "b3", "Pb1", "Pb2")
        ])[None, :], (P, 1))
    common = dict(
        W1=np.asarray(inputs["W1"], np.float32),
        W2=np.asarray(inputs["W2"], np.float32),
        W3=np.asarray(inputs["W3"], np.float32),
        Pw1=np.asarray(inputs["Pw1"], np.float32),
        Pw2=np.asarray(inputs["Pw2"], np.float32),
        brep=brep,
    )
    maps = []
    for c in range(N_CORES):
        xc = xn[c * rpc:(c + 1) * rpc]
        maps.append(dict(
            xT=np.ascontiguousarray(xc.T),
            rwt=pl["rw"][c],
            sidx=pl["sidx"][c],
            **common,
        ))
    return maps


def build_all(inputs, n=None, in_d=IN_D, **bkw):
    x = np.asarray(inputs["x"])
    n = x.shape[0] if n is None else n
    pl = _plan(np.asarray(inputs["edge_index"]), n)
    nc = _build(pl, in_d=in_d, **bkw)
    maps = _in_maps(inputs, pl, in_d=in_d)
    return nc, maps, pl


def postprocess(results, pl):
    z_new = np.concatenate([results[c]["z"] for c in range(N_CORES)], axis=0)
    return np.ascontiguousarray(z_new[pl["new_id"]]).astype(np.float32)


def kernel(**inputs) -> np.ndarray:
    from concourse.bass_utils import run_bass_kernel_spmd
    nc, maps, pl = build_all(inputs)
    res = run_bass_kernel_spmd(nc, maps, core_ids=list(range(N_CORES)))
    return postprocess(res.results, pl)
